# revision 1
# baseline (speedup 1.0000x reference)
"""GroupedQueryAttention on 8 Trainium2 NeuronCores.

Sharding: 4-way tensor-parallel over heads x 2-way data-parallel over batch.
Core c handles batch c//4 and head-group g=c%4 (q heads 8g..8g+7, kv heads
2g, 2g+1); o-proj is row-sharded so the host sums 4 partials per batch.

Per-core dataflow (matmuls in fp32r), fully fused over 512-token slices --
causality means slice ts only needs q/k/v from slices <= ts:
  stage 1 (per ts): fused QKV projection producing transposed layouts
           qT [64, 8, 512] (head-dim on partitions), kT [64, 2, T] grows
           incrementally, v via PE-transpose into v_aug [128, 2, kt, 65]
           (ones column -> softmax denominator comes free out of mm2)
  stage 2 (per ts, h): scoresT [k,q] blocks = kT_chunk.T @ qT; exp on ACT
           (no max-subtraction: scores ~ N(0,1), exp cannot overflow);
           causal via column-range restriction + one triangular mask mul
           on the diagonal 128x128 block; ctx_psum [65, 512] accumulates
           v_aug.T @ attnT over k-tiles; normalize with DVE reciprocal +
           K=1 PE broadcast matmul + DVE mul
  stage 3 (per ts, interleaved into next slice's stage 2): out partial
           tile [128, 512] = ctx_chunk.T @ wo_chunk, DMA out
"""
import sys

sys.path.insert(0, "/opt/trn_rl_repo")

import numpy as np

import concourse.bass as bass  # noqa: F401
import concourse.mybir as mybir
import concourse.tile as tile
from concourse import bacc
from concourse.bass_utils import run_bass_kernel_spmd
from concourse.masks import make_identity

F32 = mybir.dt.float32
F32R = mybir.dt.float32r
AF = mybir.ActivationFunctionType

N_CORES = 8
B, T, D = 2, 2048, 2048
H, KVH, HD = 32, 8, 64
H_L = 8                       # q heads per core
KV_L = 2                      # kv heads per core
QKV_COLS = (H_L + 2 * KV_L) * HD  # 768
NCH = QKV_COLS // 128         # 6 projection chunks
TS = 512
NTS = T // TS                 # 4 token slices
NDT = D // 128                # 16 contraction tiles
SCALE = HD ** -0.5


def _build():
    nc = bacc.Bacc("TRN2", target_bir_lowering=False, debug=False,
                   num_devices=N_CORES)
    xT = nc.dram_tensor("xT", [D, T], F32R, kind="ExternalInput").ap()
    wqkv = nc.dram_tensor("wqkv", [D, QKV_COLS], F32R, kind="ExternalInput").ap()
    wo = nc.dram_tensor("wo", [H_L * HD, D], F32R, kind="ExternalInput").ap()
    tri = nc.dram_tensor("tri", [128, 128], F32, kind="ExternalInput").ap()
    out = nc.dram_tensor("out", [T, D], F32, kind="ExternalOutput").ap()

    with tile.TileContext(nc) as tc:
        with tc.tile_pool(name="const", bufs=1) as cpool, \
             tc.tile_pool(name="xp", bufs=6) as xpool, \
             tc.tile_pool(name="qt", bufs=1) as qtpool, \
             tc.tile_pool(name="vt", bufs=1) as vtpool, \
             tc.tile_pool(name="ctx", bufs=2) as ctxpool, \
             tc.tile_pool(name="persist", bufs=1) as ppool, \
             tc.tile_pool(name="attn", bufs=3) as atpool, \
             tc.tile_pool(name="small", bufs=2) as smpool, \
             tc.tile_pool(name="outp", bufs=2) as outpool, \
             tc.tile_pool(name="ps128", bufs=4, space="PSUM") as pp128, \
             tc.tile_pool(name="psctx", bufs=2, space="PSUM") as ppctx, \
             tc.tile_pool(name="psmisc", bufs=2, space="PSUM") as ppmisc:

            # ---- persistent / constant tiles ----
            kT_sb = ppool.tile([64, KV_L, T], F32R, tag="kT")
            vaug_sb = ppool.tile([128, KV_L, NTS * 4, HD + 1], F32R, tag="vaug")
            wqkv_sb = cpool.tile([128, NCH, NDT, 128], F32R)
            wo_sb = cpool.tile([128, 4, D], F32R)
            tri_sb = cpool.tile([128, 128], F32)
            ident = cpool.tile([64, 64], F32)
            ones_f = cpool.tile([128, 1], F32)
            ones_row = cpool.tile([1, 64], F32R)

            def dma_xt(ts):
                tiles = []
                for qtr in range(4):
                    xt = xpool.tile([128, NDT // 4, TS], F32R, tag="xt",
                                    name=f"xt_{ts}_{qtr}")
                    r0 = qtr * (D // 4)
                    nc.sync.dma_start(
                        xt[:],
                        xT[r0:r0 + D // 4, ts * TS:(ts + 1) * TS]
                        .rearrange("(n p) m -> p n m", p=128))
                    tiles.append(xt)
                return tiles

            def dma_wqkv(ch):
                nc.sync.dma_start(
                    wqkv_sb[:, ch],
                    wqkv[:, ch * 128:(ch + 1) * 128]
                    .rearrange("(n p) m -> p n m", p=128))

            dma_wqkv(0)
            xt_cur = dma_xt(0)
            for ch in range(1, NCH):
                dma_wqkv(ch)
            nc.sync.dma_start(tri_sb[:], tri[:])
            for j in range(4):
                nc.sync.dma_start(wo_sb[:, j], wo[j * 128:(j + 1) * 128, :])
            make_identity(nc, ident[:])
            nc.vector.memset(ones_f[:], 1.0)
            nc.vector.tensor_copy(ones_row[:],
                                  ones_f[0:1, 0:1].broadcast_to([1, 64]))
            nc.vector.tensor_copy(
                vaug_sb[:, :, :, HD:HD + 1],
                ones_f[:, 0:1].broadcast_to([128, KV_L, NTS * 4, 1]))

            ctx_prev = None  # (ctx_tile, ts) pending o-projection

            def oproj_tile(ctx_t, ts, tt):
                """One 128-token row block of the output projection."""
                for ds in range(D // TS):
                    op = pp128.tile([128, TS], F32, tag="mm",
                                    name=f"op_{ts}_{tt}_{ds}")
                    for j in range(4):
                        nc.tensor.matmul(
                            op[:],
                            ctx_t[:, j, tt * 128:(tt + 1) * 128],
                            wo_sb[:, j, ds * TS:(ds + 1) * TS],
                            start=(j == 0), stop=(j == 3))
                    ot = outpool.tile([128, TS], F32, tag="ot",
                                      name=f"ot_{ts}_{tt}_{ds}")
                    nc.vector.tensor_copy(ot[:], op[:])
                    r0 = ts * TS + tt * 128
                    nc.sync.dma_start(
                        out[r0:r0 + 128, ds * TS:(ds + 1) * TS], ot[:])

            for ts in range(NTS):
                # ---- stage 1: QKV projection for slice ts ----
                qT_t = qtpool.tile([64, H_L, TS], F32R, tag="qT",
                                   name=f"qT_{ts}")
                vT_t = vtpool.tile([64, KV_L, TS], F32, tag="vT",
                                   name=f"vT_{ts}")
                xt_next = dma_xt(ts + 1) if ts + 1 < NTS else None
                for ch in range(NCH):
                    ps = pp128.tile([128, TS], F32, tag="mm",
                                    name=f"qkv_{ts}_{ch}")
                    for dt in range(NDT):
                        nc.tensor.matmul(
                            ps[:],
                            wqkv_sb[:, ch, dt, :],
                            xt_cur[dt // 4][:, dt % 4, :],
                            start=(dt == 0), stop=(dt == NDT - 1))
                    if ch < 4:
                        nc.vector.tensor_copy(qT_t[:, 2 * ch, :], ps[0:64, :])
                        nc.vector.tensor_copy(qT_t[:, 2 * ch + 1, :],
                                              ps[64:128, :])
                    elif ch == 4:
                        tsl = slice(ts * TS, (ts + 1) * TS)
                        nc.vector.tensor_copy(kT_sb[:, 0, tsl], ps[0:64, :])
                        nc.vector.tensor_copy(kT_sb[:, 1, tsl], ps[64:128, :])
                    else:
                        nc.vector.tensor_copy(vT_t[:, 0, :], ps[0:64, :])
                        nc.vector.tensor_copy(vT_t[:, 1, :], ps[64:128, :])
                xt_cur = xt_next
                for kv in range(KV_L):
                    for blk in range(4):
                        tp = ppmisc.tile([128, 64], F32, tag="misc",
                                         name=f"vt_{ts}_{kv}_{blk}")
                        nc.tensor.transpose(
                            tp[:], vT_t[:, kv, blk * 128:(blk + 1) * 128],
                            ident[:])
                        nc.vector.tensor_copy(
                            vaug_sb[:, kv, 4 * ts + blk, 0:HD], tp[:])

                # ---- stage 2 (slice ts) with stage 3 (slice ts-1) woven in
                ctx_t = ctxpool.tile([128, 4, TS], F32R, tag="ctx",
                                     name=f"ctx_{ts}")
                n_kt = 4 * (ts + 1)
                for h in range(H_L):
                    kv = h // 4
                    ctx_ps = ppctx.tile([HD + 1, TS], F32, tag="ctx",
                                        name=f"cps_{ts}_{h}")
                    for kt in range(n_kt):
                        d = kt - 4 * ts
                        c0 = 128 * d if d >= 0 else 0
                        sc = pp128.tile([128, TS], F32, tag="mm",
                                        name=f"sc_{ts}_{h}_{kt}")
                        nc.tensor.matmul(
                            sc[:, c0:],
                            kT_sb[:, kv, kt * 128:(kt + 1) * 128],
                            qT_t[:, h, c0:],
                            start=True, stop=True)
                        at = atpool.tile([128, TS], F32R, tag="at",
                                         name=f"at_{ts}_{h}_{kt}")
                        nc.scalar.activation(at[:, c0:], sc[:, c0:], AF.Exp,
                                             scale=SCALE)
                        if d >= 0:
                            nc.vector.tensor_mul(at[:, c0:c0 + 128],
                                                 at[:, c0:c0 + 128], tri_sb[:])
                        nc.tensor.matmul(
                            ctx_ps[:, c0:], vaug_sb[:, kv, kt, :], at[:, c0:],
                            start=(kt == 0), stop=(kt == n_kt - 1))
                    rc = smpool.tile([1, TS], F32R, tag="rc", name=f"rc_{ts}_{h}")
                    with nc.allow_low_precision(reason="softmax recip"):
                        nc.vector.reciprocal(rc[:], ctx_ps[HD:HD + 1, :])
                    bc = ppmisc.tile([64, TS], F32, tag="misc",
                                     name=f"bc_{ts}_{h}")
                    nc.tensor.matmul(bc[:], ones_row[:], rc[:],
                                     start=True, stop=True)
                    rc64 = smpool.tile([64, TS], F32, tag="rc64",
                                       name=f"rc64_{ts}_{h}")
                    nc.vector.tensor_copy(rc64[:], bc[:])
                    p0 = 64 * (h % 2)
                    nc.vector.tensor_mul(
                        ctx_t[p0:p0 + 64, h // 2, :], ctx_ps[0:HD, :], rc64[:])
                    if ctx_prev is not None and h % 2 == 1:
                        oproj_tile(ctx_prev[0], ctx_prev[1], h // 2)
                ctx_prev = (ctx_t, ts)

            for tt in range(4):
                oproj_tile(ctx_prev[0], ctx_prev[1], tt)

    nc.compile()
    return nc


_NC = None


def _get_nc():
    global _NC
    if _NC is None:
        _NC = _build()
    return _NC


def _make_in_maps(x, wq, wkv, wo):
    x = np.asarray(x, dtype=np.float32)
    wq = np.asarray(wq, dtype=np.float32)
    wkv = np.asarray(wkv, dtype=np.float32)
    wo = np.asarray(wo, dtype=np.float32)

    xTb = [np.ascontiguousarray(x[b].T) for b in range(B)]
    tri = np.triu(np.ones((128, 128), dtype=np.float32))

    in_maps = []
    for c in range(N_CORES):
        b, g = c // 4, c % 4
        qcols = slice(g * H_L * HD, (g + 1) * H_L * HD)        # 512 cols
        kcols = slice(g * KV_L * HD, (g + 1) * KV_L * HD)      # 128 cols
        vcols = slice(KVH * HD + g * KV_L * HD,
                      KVH * HD + (g + 1) * KV_L * HD)
        wqkv_c = np.ascontiguousarray(
            np.concatenate([wq[:, qcols], wkv[:, kcols], wkv[:, vcols]],
                           axis=1))
        wo_c = np.ascontiguousarray(wo[qcols, :])
        in_maps.append({"xT": xTb[b], "wqkv": wqkv_c, "wo": wo_c, "tri": tri})
    return in_maps


def kernel(x, wq, wkv, wo):
    in_maps = _make_in_maps(x, wq, wkv, wo)
    res = run_bass_kernel_spmd(_get_nc(), in_maps, list(range(N_CORES)))
    acc = np.zeros((B, T, D), dtype=np.float64)
    for c, r in enumerate(res.results):
        acc[c // 4] += r["out"]
    return acc.astype(np.float32)



# revision 15
# speedup vs baseline: 1.7077x; 1.7077x over previous
"""GroupedQueryAttention on 8 Trainium2 NeuronCores.

Sharding: 4-way tensor-parallel over heads x 2-way data-parallel over batch.
Core c handles batch c//4 and head-group g=c%4 (q heads 8g..8g+7, kv heads
2g, 2g+1); o-proj is row-sharded so the host sums 4 partials per batch.

All matmuls run in bf16 (fp32 PSUM accumulate); rel tolerance 2e-2 leaves
~5x margin.  Per-core dataflow, fused over 512-token slices (causality:
slice ts only needs k/v from slices <= ts):
  stage 1 (per ts): fused QKV projection in transposed layout; v chunk
           first, then k, then q, so the PE-transposes of v into
           v_aug [128, kv, kt, 65] (ones column -> softmax denominator
           falls out of mm2) overlap the q matmuls.
  stage 2 (per ts): flat software-pipelined (h, kt) loop: scoresT block
           matmul -> exp on ACT (bf16 out, no max-subtraction; scores
           ~N(0,1)) -> triangular mask mul on Pool for the diagonal
           block -> mm2 accumulate [65, 512]; mm2(kt) is issued after
           scores(kt+1) so the PE never waits on the ACT engine.
           o-proj of slice ts-1 is woven in at every odd head.
  normalize (deferred, issued after next slice's QKV): batched
           reciprocal_approx_fast over all 8 heads' denominators, K=8
           select-matmul broadcasts across partitions, one DVE mul per
           128-partition chunk -> normalized ctx in bf16.
  stage 3: out partial tile [128, 512] = ctx_chunk.T @ wo_chunk, bf16
           DMA out; host upcasts and reduces the 4 partials.
"""
import sys

sys.path.insert(0, "/opt/trn_rl_repo")

import numpy as np
import ml_dtypes

import concourse.bass as bass  # noqa: F401
import concourse.mybir as mybir
import concourse.tile as tile
from concourse import bacc
from concourse.bass_utils import run_bass_kernel_spmd
from concourse.masks import make_identity

F32 = mybir.dt.float32
BF16 = mybir.dt.bfloat16
AF = mybir.ActivationFunctionType
NP_BF16 = ml_dtypes.bfloat16

N_CORES = 8
B, T, D = 2, 2048, 2048
H, KVH, HD = 32, 8, 64
H_L = 8                       # q heads per core
KV_L = 2                      # kv heads per core
QKV_COLS = (H_L + 2 * KV_L) * HD  # 768
NCH = QKV_COLS // 128         # 6 projection chunks
TS = 512
NTS = T // TS                 # 4 token slices
NDT = D // 128                # 16 contraction tiles
SCALE = HD ** -0.5
CH_ORDER = (5, 4, 0, 1, 2, 3)  # v, k, then q chunks


def _build():
    nc = bacc.Bacc("TRN2", target_bir_lowering=False, debug=False,
                   num_devices=N_CORES)
    xT = nc.dram_tensor("xT", [128, NDT, T], BF16, kind="ExternalInput").ap()
    wqkv = nc.dram_tensor("wqkv", [128, NCH, NDT, 128], BF16,
                          kind="ExternalInput").ap()
    wo = nc.dram_tensor("wo", [128, 4, D], BF16, kind="ExternalInput").ap()
    tri = nc.dram_tensor("tri", [128, 128], BF16, kind="ExternalInput").ap()
    out = nc.dram_tensor("out", [T, D], BF16, kind="ExternalOutput").ap()

    with tile.TileContext(nc) as tc, \
         nc.allow_low_precision(reason="bf16 kernel, tol 2e-2"):
        with tc.tile_pool(name="const", bufs=1) as cpool, \
             tc.tile_pool(name="xp", bufs=8) as xpool, \
             tc.tile_pool(name="qt", bufs=2) as qtpool, \
             tc.tile_pool(name="vt", bufs=2) as vtpool, \
             tc.tile_pool(name="ctxr", bufs=2) as crpool, \
             tc.tile_pool(name="ctx", bufs=2) as ctxpool, \
             tc.tile_pool(name="persist", bufs=1) as ppool, \
             tc.tile_pool(name="attn", bufs=4) as atpool, \
             tc.tile_pool(name="small", bufs=2) as smpool, \
             tc.tile_pool(name="outp", bufs=3) as outpool, \
             tc.tile_pool(name="ps128", bufs=4, space="PSUM") as pp128, \
             tc.tile_pool(name="psctx", bufs=2, space="PSUM") as ppctx, \
             tc.tile_pool(name="psmisc", bufs=2, space="PSUM") as ppmisc:

            # ---- persistent / constant tiles ----
            kT_sb = ppool.tile([128, T], BF16, tag="kT")
            vaug_sb = ppool.tile([128, KV_L, NTS * 4, HD + 1], BF16,
                                 tag="vaug")
            wqkv_sb = cpool.tile([128, NCH, NDT, 128], BF16)
            wo_sb = cpool.tile([128, 4, D], BF16)
            tri_sb = cpool.tile([128, 128], BF16)
            ident = cpool.tile([128, 64], BF16)  # identity in both halves
            ones_f = cpool.tile([128, 1], BF16)
            # sel_l[:, c, :]: row 32c is ones -> broadcasts den row 32c
            # (head c or c+4, depending on rhs free-slot) to 64 partitions
            sel_l = cpool.tile([128, 4, 64], BF16)

            def dma_xt(ts):
                tiles = []
                for qtr in range(4):
                    xt = xpool.tile([128, 4, TS], BF16, tag="xt",
                                    name=f"xt_{ts}_{qtr}")
                    nc.sync.dma_start(
                        xt[:],
                        xT[:, qtr * 4:(qtr + 1) * 4,
                           ts * TS:(ts + 1) * TS])
                    tiles.append(xt)
                return tiles

            nc.sync.dma_start(wqkv_sb[:], wqkv[:])
            xt_cur = dma_xt(0)
            nc.sync.dma_start(tri_sb[:], tri[:])
            nc.sync.dma_start(wo_sb[:], wo[:])
            make_identity(nc, ident[0:64, :])
            make_identity(nc, ident[64:128, :])
            nc.gpsimd.memset(ones_f[:], 1.0)
            nc.gpsimd.memset(sel_l[:], 0.0)
            for c in range(4):
                nc.gpsimd.memset(sel_l[32 * c:32 * c + 1, c, :], 1.0)
            nc.vector.tensor_copy(
                vaug_sb[:, :, :, HD:HD + 1],
                ones_f[:, 0:1].broadcast_to([128, KV_L, NTS * 4, 1]))

            ctx_prev = None  # (ctx_tile, ts) pending o-projection

            def oproj_tile(ctx_t, ts, tt):
                """One 128-token row block of the output projection."""
                for ds in range(D // TS):
                    op = pp128.tile([128, TS], F32, tag="mm",
                                    name=f"op_{ts}_{tt}_{ds}")
                    for j in range(4):
                        nc.tensor.matmul(
                            op[:],
                            ctx_t[:, j, tt * 128:(tt + 1) * 128],
                            wo_sb[:, j, ds * TS:(ds + 1) * TS],
                            start=(j == 0), stop=(j == 3))
                    ot = outpool.tile([128, TS], BF16, tag="ot",
                                      name=f"ot_{ts}_{tt}_{ds}")
                    nc.vector.tensor_copy(ot[:], op[:])
                    r0 = ts * TS + tt * 128
                    nc.sync.dma_start(
                        out[r0:r0 + 128, ds * TS:(ds + 1) * TS], ot[:])

            def normalize(ctx_raw_t, den_t, ctx_t, ts):
                """Batched softmax denominators -> normalized bf16 ctx."""
                rc = smpool.tile([128, 2, TS], F32, tag="rc", name=f"rc_{ts}")
                nc.vector.reciprocal_approx_fast(rc[:], den_t[:])
                rc16 = smpool.tile([128, 2, TS], BF16, tag="rc16",
                                   name=f"rc16_{ts}")
                nc.vector.tensor_copy(rc16[:], rc[:])
                for c in range(4):
                    rcb = ppmisc.tile([128, TS], F32, tag="misc",
                                      name=f"rcb_{ts}_{c}")
                    nc.tensor.matmul(rcb[0:64, :], sel_l[:, c, :],
                                     rc16[:, 0, :], start=True, stop=True)
                    nc.tensor.matmul(rcb[64:128, :], sel_l[:, c, :],
                                     rc16[:, 1, :], start=True, stop=True)
                    nc.vector.tensor_mul(ctx_t[:, c, :],
                                         ctx_raw_t[:, c, :], rcb[:])

            norm_pend = None  # args for deferred normalize

            for ts in range(NTS):
                # ---- stage 1: QKV projection for slice ts ----
                qT2 = qtpool.tile([128, 4, TS], BF16, tag="qT",
                                  name=f"qT_{ts}")
                vT_t = vtpool.tile([128, TS], BF16, tag="vT",
                                   name=f"vT_{ts}")
                xt_next = dma_xt(ts + 1) if ts + 1 < NTS else None
                for ch in CH_ORDER:
                    ps = pp128.tile([128, TS], F32, tag="mm",
                                    name=f"qkv_{ts}_{ch}")
                    for dt in range(NDT):
                        nc.tensor.matmul(
                            ps[:],
                            wqkv_sb[:, ch, dt, :],
                            xt_cur[dt // 4][:, dt % 4, :],
                            start=(dt == 0), stop=(dt == NDT - 1))
                    if ch == 5:
                        nc.vector.tensor_copy(vT_t[:], ps[:])
                    elif ch == 4:
                        nc.vector.tensor_copy(
                            kT_sb[:, ts * TS:(ts + 1) * TS], ps[:])
                        for kv in range(KV_L):
                            for blk in range(4):
                                tp = ppmisc.tile([128, 64], BF16, tag="misc",
                                                 name=f"vt_{ts}_{kv}_{blk}")
                                nc.tensor.transpose(
                                    tp[:],
                                    vT_t[64 * kv:64 * kv + 64,
                                         blk * 128:(blk + 1) * 128],
                                    ident[64 * kv:64 * kv + 64, :])
                                nc.vector.tensor_copy(
                                    vaug_sb[:, kv, 4 * ts + blk, 0:HD], tp[:])
                    else:
                        nc.vector.tensor_copy(qT2[:, ch, :], ps[:])
                xt_cur = xt_next

                # deferred normalize of the previous slice (keeps the PE
                # fed with QKV matmuls while the DVE recip chain runs)
                if norm_pend is not None:
                    normalize(*norm_pend)
                    norm_pend = None

                # ---- stage 2: attention, software-pipelined; o-proj of
                # slice ts-1 woven in at odd heads ----
                ctx_raw = crpool.tile([128, 4, TS], BF16, tag="ctxr",
                                      name=f"ctxr_{ts}")
                ctx_t = ctxpool.tile([128, 4, TS], BF16, tag="ctx",
                                     name=f"ctx_{ts}")
                # head h's denominator row lives at partition 32*(h%4),
                # free-slot h//4 (partition offsets must be 32-aligned);
                # memset to 1.0 so untouched partitions can't feed NaN
                # into the select matmul (0 * nan = nan)
                den_t = smpool.tile([128, 2, TS], F32, tag="den",
                                    name=f"den_{ts}")
                nc.gpsimd.memset(den_t[:], 1.0)
                n_kt = 4 * (ts + 1)
                pend = None  # (kv, kt, at, c0, ctx_ps, start, stop, h)

                def flush(p):
                    kv, kt, at, c0, cps, st, sp, h = p
                    nc.tensor.matmul(
                        cps[:, c0:], vaug_sb[:, kv, kt, :], at[:, c0:],
                        start=st, stop=sp)
                    if sp:
                        r0 = 32 * (h % 4)
                        nc.vector.tensor_copy(den_t[r0:r0 + 1, h // 4, :],
                                              cps[HD:HD + 1, :])
                        nc.vector.tensor_copy(
                            ctx_raw[(h // 4) * 64:(h // 4) * 64 + 64,
                                    h % 4, :],
                            cps[0:HD, :])

                for h in range(H_L):
                    kv = h // 4
                    p0 = (h // 4) * 64  # q head base partition == kv base
                    ctx_ps = ppctx.tile([HD + 1, TS], F32, tag="ctx",
                                        name=f"cps_{ts}_{h}")
                    if ctx_prev is not None and h % 2 == 1:
                        oproj_tile(ctx_prev[0], ctx_prev[1], h // 2)
                    for kt in range(n_kt):
                        d = kt - 4 * ts
                        c0 = 128 * d if d >= 0 else 0
                        sc = pp128.tile([128, TS], F32, tag="mm",
                                        name=f"sc_{ts}_{h}_{kt}")
                        nc.tensor.matmul(
                            sc[:, c0:],
                            kT_sb[64 * kv:64 * kv + 64,
                                  kt * 128:(kt + 1) * 128],
                            qT2[p0:p0 + 64, h % 4, c0:],
                            start=True, stop=True)
                        at = atpool.tile([128, TS], BF16, tag="at",
                                         name=f"at_{ts}_{h}_{kt}")
                        nc.scalar.activation(at[:, c0:], sc[:, c0:], AF.Exp,
                                             scale=SCALE)
                        if d >= 0:
                            nc.gpsimd.tensor_mul(at[:, c0:c0 + 128],
                                                 at[:, c0:c0 + 128],
                                                 tri_sb[:])
                        if pend is not None:
                            flush(pend)
                        pend = (kv, kt, at, c0, ctx_ps,
                                kt == 0, kt == n_kt - 1, h)
                flush(pend)
                norm_pend = (ctx_raw, den_t, ctx_t, ts)
                ctx_prev = (ctx_t, ts)

            normalize(*norm_pend)
            for tt in range(4):
                oproj_tile(ctx_prev[0], ctx_prev[1], tt)

    nc.compile()
    return nc


_NC = None


def _get_nc():
    global _NC
    if _NC is None:
        _NC = _build()
    return _NC


def _make_in_maps(x, wq, wkv, wo):
    x = np.asarray(x, dtype=np.float32)
    wq = np.asarray(wq, dtype=np.float32)
    wkv = np.asarray(wkv, dtype=np.float32)
    wo = np.asarray(wo, dtype=np.float32)

    # x[b].T tiled [128, NDT, T] so each slice DMA is 1KB-per-partition runs
    xTb = []
    for b in range(B):
        xt = np.ascontiguousarray(
            x[b].T.reshape(NDT, 128, T).transpose(1, 0, 2).astype(NP_BF16))
        xTb.append(xt)
    tri = np.triu(np.ones((128, 128), dtype=np.float32)).astype(NP_BF16)

    # head order within a core: chunk c holds heads c (parts 0-63) and
    # c+4 (parts 64-127), so each q head's base partition matches its kv
    # head's base partition in kT_sb
    perm = [0, 4, 1, 5, 2, 6, 3, 7]
    in_maps = []
    for c in range(N_CORES):
        b, g = c // 4, c % 4
        h0 = g * H_L                                           # first q head
        qblocks = [wq[:, (h0 + p) * HD:(h0 + p + 1) * HD] for p in perm]
        kcols = slice(g * KV_L * HD, (g + 1) * KV_L * HD)      # 128 cols
        vcols = slice(KVH * HD + g * KV_L * HD,
                      KVH * HD + (g + 1) * KV_L * HD)
        wqkv_c = np.concatenate(qblocks + [wkv[:, kcols], wkv[:, vcols]],
                                axis=1)                         # [D, 768]
        # -> [128, NCH, NDT, 128] partition-major for one contiguous DMA
        wqkv_c = np.ascontiguousarray(
            wqkv_c.reshape(NDT, 128, NCH, 128).transpose(1, 2, 0, 3)
            .astype(NP_BF16))
        wo_rows = np.concatenate(
            [wo[(h0 + p) * HD:(h0 + p + 1) * HD, :] for p in perm], axis=0)
        wo_c = np.ascontiguousarray(
            wo_rows.reshape(4, 128, D).transpose(1, 0, 2)
            .astype(NP_BF16))                                   # [128, 4, D]
        in_maps.append({"xT": xTb[b], "wqkv": wqkv_c, "wo": wo_c, "tri": tri})
    return in_maps


def kernel(x, wq, wkv, wo):
    in_maps = _make_in_maps(x, wq, wkv, wo)
    res = run_bass_kernel_spmd(_get_nc(), in_maps, list(range(N_CORES)))
    acc = np.zeros((B, T, D), dtype=np.float32)
    for c, r in enumerate(res.results):
        acc[c // 4] += r["out"].astype(np.float32)
    return acc
